# revision 1
# baseline (speedup 1.0000x reference)
"""TopK autoencoder (SAE) kernel for Trainium2, 8 NeuronCores, data-parallel over batch.

Per core (512 rows):
  Phase 1: streaming encoder projT[f,b] via an fp16 two-term split
           (hi = fp16(v), lo = fp16((v - hi) * 2^12)): proj = wh@xh +
           2^-12*(wh@xl' + wl'@xh), exact to ~2^-22 (top-k set equality vs
           the fp32 reference needs ~1e-6 proj accuracy — any selection swap
           costs ~5-10%% output error vs the 2%% gate). Spill projT fp32 to
           DRAM, PE-transpose blocks to [b,f], extract top-8-per-superchunk
           candidate arrays (max8) for main (sc=128) and dead-masked (sc=32).
  Phase 1.5: per-row exact k-th-largest thresholds via midpoint bisection on
           the candidate arrays, implemented entirely on the ACT engine
           (Sign+accum count -> Sign step -> Identity midpoint update) so the
           four 128-row blocks' bisections pipeline behind phase 2 decodes.
  Phase 2: per 256-row pair of blocks: stream projT back + bf16 lookup
           (two passes total), build sparse S^T = x * (x >= t) in [f,b]
           layout as bf16 (DVE only; ACT is busy bisecting ahead), dense bf16
           decoder matmuls accumulating main+dead into all 8 PSUM banks, add
           enc_bias to main recon. Decoder precision margin is large (bf16
           gives ~2e-3 rel vs the 2e-2 gate); fp8 would fail.
"""
import numpy as np

B, E, F = 4096, 1024, 32768
NCORES = 8
BL = B // NCORES           # 512 rows per core
TOPK, DEAD_TOPK = 64, 512
DEAD_CUTOFF = 50000

FBLK = 512                 # phase-1 f-block
SC_MAIN, SC_DEAD = 128, 32
TM_LO, TM_HI = 3.65, 4.50      # bisection brackets (calibrated, with margin)
TD_LO, TD_HI = 2.30, 2.90
BIS_ITERS = 23
SPLIT_BITS = 11                # fp32r hi/lo mantissa split
FT_FUSE = 4                    # phase-2 f-tiles per iteration

_CACHED = {}


def _build(f_total, phases=("p1", "p15", "p2"), enc_products=3, extract=True, fuse1024=False,
           ft_fuse=FT_FUSE, p2_bufs=3, p2c_bufs=None, bis_iters=BIS_ITERS,
           brackets=(TM_LO, TM_HI, TD_LO, TD_HI)):
    tm_lo, tm_hi, td_lo, td_hi = brackets
    if p2c_bufs is None:
        p2c_bufs = 2 if ft_fuse <= 4 else 1
    import concourse.bass as bass
    from concourse import bacc
    import concourse.mybir as mybir
    import concourse.tile as tile
    from concourse.masks import make_identity

    F32 = mybir.dt.float32
    F16 = mybir.dt.float16
    BF16 = mybir.dt.bfloat16
    SIGN = mybir.ActivationFunctionType.Sign
    IDENT = mybir.ActivationFunctionType.Identity

    n_fblk = f_total // FBLK
    n_ftile = f_total // 128
    ncm = (f_total // SC_MAIN) * 8     # 2048 main candidates
    ncd = (f_total // SC_DEAD) * 8     # 8192 dead candidates
    n_it64 = n_ftile // ft_fuse        # phase-2 iterations per row-pair

    nc = bacc.Bacc(None, target_bir_lowering=False)

    whT = nc.dram_tensor("whT", [E, f_total], F16, kind="ExternalInput")
    wlT = nc.dram_tensor("wlT", [E, f_total], F16, kind="ExternalInput")
    xhT = nc.dram_tensor("xhT", [E, BL], F16, kind="ExternalInput")
    xlT = nc.dram_tensor("xlT", [E, BL], F16, kind="ExternalInput")
    lookup_bf = nc.dram_tensor("lookup_bf", [f_total, E], BF16, kind="ExternalInput")
    pen_row = nc.dram_tensor("pen_row", [1, f_total], F32, kind="ExternalInput")
    pen_pt = nc.dram_tensor("pen_pt", [128, f_total // 128], F32, kind="ExternalInput")
    bias_row = nc.dram_tensor("bias_row", [1, E], F32, kind="ExternalInput")

    out_main = nc.dram_tensor("out_main", [BL, E], F32, kind="ExternalOutput")
    out_dead = nc.dram_tensor("out_dead", [BL, E], F32, kind="ExternalOutput")

    projT_dram = nc.dram_tensor("projT_dram", [f_total, BL], F32)
    t_dram = nc.dram_tensor("t_dram", [2, BL], F32)
    md8_dram = nc.dram_tensor("md8_dram", [4, 128, ncd], F32)

    def bcast(ap_row):
        # [1, n] dram AP -> partition-broadcast to 128
        return bass.AP(tensor=ap_row.tensor, offset=ap_row.offset,
                       ap=[[0, 128]] + list(ap_row.ap[1:]))

    thr_m = float(2 * TOPK - ncm)
    thr_d = float(2 * DEAD_TOPK - ncd)
    w0_m = (tm_hi - tm_lo) / 2.0
    w0_d = (td_hi - td_lo) / 2.0

    with tile.TileContext(nc) as tc:
        eng = [nc.sync, nc.scalar, nc.gpsimd]

        with tc.tile_pool(name="const", bufs=1) as const_pool:
            ident = const_pool.tile([128, 128], F32)
            make_identity(nc, ident)

            # main candidate arrays (persist through phase 1 + 1.5); dead ones
            # are staged to DRAM (too big for SBUF at sc=32)
            mm8 = [const_pool.tile([128, ncm], F32, name=f"mm8_{rt}") for rt in range(4)]

            # ---------------- PHASE 1 ----------------
            with (
                tc.tile_pool(name="p1w", bufs=2) as p1w,
                tc.tile_pool(name="p1x", bufs=1) as p1x,
                tc.tile_pool(name="p1s", bufs=3) as p1s,
                tc.tile_pool(name="p1b", bufs=3) as p1b,
                tc.tile_pool(name="psA", bufs=1, space="PSUM") as psA,
                tc.tile_pool(name="psB", bufs=1, space="PSUM") as psB,
            ):
                # xboth = [xh | xl*2^12] along free axis: the fused moving
                # operand shares the wh stationary load across both products.
                xboth = p1x.tile([128, 8, 2 * BL], F16)
                nc.sync.dma_start(xboth[:, :, 0:BL],
                                  xhT.rearrange("(c p) b -> p c b", p=128))
                nc.sync.dma_start(xboth[:, :, BL:2 * BL],
                                  xlT.rearrange("(c p) b -> p c b", p=128))

                for blk in range(n_fblk if "p1" in phases else 0):
                    f0 = blk * FBLK
                    wh_blk = p1w.tile([128, 8, FBLK], F16, name="wh_blk")
                    wl_blk = p1w.tile([128, 8, FBLK], F16, name="wl_blk")
                    eng[blk % 2].dma_start(
                        wh_blk, whT[:, f0:f0 + FBLK].rearrange("(c p) f -> p c f", p=128))
                    if enc_products >= 3:
                        eng[(blk + 1) % 2].dma_start(
                            wl_blk, wlT[:, f0:f0 + FBLK].rearrange("(c p) f -> p c f", p=128))

                    pen_b = p1b.tile([128, FBLK], F32, name="pen_b")
                    nc.gpsimd.dma_start(pen_b, bcast(pen_row[:, f0:f0 + FBLK]))

                    # psB quadrant accumulators [b-tile, FBLK]
                    pB = [psB.tile([128, FBLK], F32, name=f"pB{bj}", tag=f"pB{bj}") for bj in range(4)]

                    nsub = FBLK // 128
                    for grp in range(nsub // 2):
                        subs = (2 * grp, 2 * grp + 1)
                        # [main | corr] accumulators: 2 banks each
                        pAB = {s: psA.tile([128, 2 * BL], F32, name=f"pAB{s % 2}",
                                           tag=f"pAB{s % 2}") for s in subs}
                        # keep main(s)+xl(s) adjacent: same stationary weights,
                        # different psum banks -> the weight reload can be
                        # skipped and banks still alternate
                        for c in range(8):
                            if c == 7 and enc_products >= 3:
                                for s in subs:
                                    ll = wl_blk[:, c, s * 128:(s + 1) * 128]
                                    nc.tensor.matmul(pAB[s][:, BL:], ll, xboth[:, c, 0:BL],
                                                     start=False, stop=False)
                            for s in subs:
                                lh = wh_blk[:, c, s * 128:(s + 1) * 128]
                                nc.tensor.matmul(pAB[s][:, 0:BL], lh, xboth[:, c, 0:BL],
                                                 start=(c == 0), stop=(c == 7))
                                if enc_products >= 2:
                                    nc.tensor.matmul(pAB[s][:, BL:], lh, xboth[:, c, BL:],
                                                     start=(c == 0), stop=(c == 7))
                            if c < 7 and enc_products >= 3:
                                for s in subs:
                                    ll = wl_blk[:, c, s * 128:(s + 1) * 128]
                                    nc.tensor.matmul(pAB[s][:, BL:], ll, xboth[:, c, 0:BL],
                                                     start=False, stop=False)
                        for s in subs:
                            pt_sb = p1s.tile([128, BL], F32, name="pt_sb")
                            if enc_products >= 2:
                                cs = p1s.tile([128, BL], F32, name="cs")
                                nc.scalar.mul(cs, pAB[s][:, BL:], float(2.0 ** -12))
                                nc.vector.tensor_tensor(pt_sb, pAB[s][:, 0:BL], cs,
                                                        mybir.AluOpType.add)
                            else:
                                nc.scalar.copy(pt_sb, pAB[s][:, 0:BL])
                            nc.sync.dma_start(
                                projT_dram[f0 + s * 128: f0 + (s + 1) * 128, :], pt_sb)
                            for bj in range(4 if extract else 0):
                                nc.tensor.transpose(
                                    pB[bj][:, s * 128:(s + 1) * 128],
                                    pt_sb[:, bj * 128:(bj + 1) * 128], ident)

                    nsl_d = FBLK // SC_DEAD
                    for bj in range(4 if extract else 0):
                        plain = p1b.tile([128, FBLK], F32, name="plain")
                        nc.scalar.copy(plain, pB[bj])
                        masked = p1b.tile([128, FBLK], F32, name="masked")
                        nc.gpsimd.tensor_tensor(masked, plain, pen_b, mybir.AluOpType.add)
                        for sl in range(FBLK // SC_MAIN):
                            nc.vector.max(
                                mm8[bj][:, (f0 // SC_MAIN + sl) * 8:(f0 // SC_MAIN + sl) * 8 + 8],
                                plain[:, sl * SC_MAIN:(sl + 1) * SC_MAIN])
                        md_stage = p1b.tile([128, nsl_d * 8], F32, name="md_stage")
                        for sl in range(nsl_d):
                            nc.vector.max(
                                md_stage[:, sl * 8:sl * 8 + 8],
                                masked[:, sl * SC_DEAD:(sl + 1) * SC_DEAD])
                        nc.sync.dma_start(
                            md8_dram[bj, :, blk * nsl_d * 8:(blk + 1) * nsl_d * 8], md_stage)

            # ---------- PHASE 1.5 (ACT-only) + PHASE 2, pipelined ----------
            with (
                tc.tile_pool(name="bis", bufs=1) as bis,
                tc.tile_pool(name="md8p", bufs=1) as md8p,
                tc.tile_pool(name="p2c", bufs=p2c_bufs) as p2c,
                tc.tile_pool(name="p2", bufs=p2_bufs) as p2,
                tc.tile_pool(name="p2o", bufs=1) as p2o,
                tc.tile_pool(name="ps2", bufs=1, space="PSUM") as ps2,
            ):
                # Bisection for all 4 row-blocks first: pure ACT chain, so the
                # ACT engine runs ahead while PE/DVE/DMA execute phase 2.
                junk_m = bis.tile([128, ncm], BF16)
                junk_d = bis.tile([128, ncd], BF16)
                if "p15" in phases:
                    cb_m = bis.tile([128, 1], F32, name="cb_m")
                    cb_d = bis.tile([128, 1], F32, name="cb_d")
                    cw_m = bis.tile([128, 1], F32, name="cw_m")
                    cw_d = bis.tile([128, 1], F32, name="cw_d")
                    nc.gpsimd.memset(cb_m, 1.0 - thr_m)
                    nc.gpsimd.memset(cb_d, 1.0 - thr_d)
                    nc.gpsimd.memset(cw_m, -(w0_m / (2.0 ** bis_iters)))
                    nc.gpsimd.memset(cw_d, -(w0_d / (2.0 ** bis_iters)))
                    NA = 4608          # ACT share of the dead-count (rt 0/1)
                    NB = ncd - NA      # DVE share
                    for rt in range(4):
                        md8_t = md8p.tile([128, ncd], F32, name="md8_t")
                        nc.sync.dma_start(md8_t, md8_dram[rt])
                        nmid_m = [bis.tile([128, 1], F32, name=f"nm_m{rt}_{i}") for i in range(2)]
                        nmid_d = [bis.tile([128, 1], F32, name=f"nm_d{rt}_{i}") for i in range(2)]
                        cnt_m = bis.tile([128, 1], F32, name=f"cnt_m{rt}")
                        cnt_d = bis.tile([128, 1], F32, name=f"cnt_d{rt}")
                        dir_m = bis.tile([128, 1], F32, name=f"dir_m{rt}")
                        dir_d = bis.tile([128, 1], F32, name=f"dir_d{rt}")
                        nc.gpsimd.memset(nmid_m[0], -(tm_lo + tm_hi) / 2.0)
                        nc.gpsimd.memset(nmid_d[0], -(td_lo + td_hi) / 2.0)
                        # NOTE: splitting the dead-count across ACT+DVE (to
                        # shrink the phase-2 startup bubble) was tried twice
                        # and produced wrong thresholds both times: a separate
                        # is_ge -> tensor_reduce miscounts (390 bad rows), and
                        # the fused tensor_scalar accum_out variant broke all
                        # split rows (accumulator semantics differ from ACT's
                        # activation accum_out). Exact counting stays on ACT.
                        split = False
                        if split:
                            # dead chain split ACT/DVE: blocks 0/1 gate the
                            # phase-2 pipeline start, so halve their latency.
                            # DVE keeps mid (not nmid); combined condition:
                            # cnt_dA + 2*cntB >= 2*DEAD_TOPK - NA
                            mid_d = [bis.tile([128, 1], F32, name=f"mi_d{rt}_{i}")
                                     for i in range(2)]
                            cntB = bis.tile([128, 1], F32, name=f"cntB{rt}")
                            selB = bis.tile([128, 1], F32, name=f"selB{rt}")
                            stB = bis.tile([128, 1], F32, name=f"stB{rt}")
                            nc.gpsimd.memset(mid_d[0], (td_lo + td_hi) / 2.0)
                        for it in range(bis_iters):
                            cur, nxt = it % 2, 1 - it % 2
                            step_m = w0_m / (2.0 ** (it + 1))
                            step_d = w0_d / (2.0 ** (it + 1))
                            nc.scalar.activation(junk_m, mm8[rt], SIGN,
                                                 bias=nmid_m[cur], scale=1.0, accum_out=cnt_m)
                            nc.scalar.activation(dir_m, cnt_m, SIGN,
                                                 bias=cb_m, scale=1.0)
                            nc.scalar.activation(nmid_m[nxt], dir_m, IDENT,
                                                 bias=nmid_m[cur], scale=-step_m)
                            if split:
                                nc.scalar.activation(junk_d[:, 0:NA], md8_t[:, 0:NA],
                                                     SIGN, bias=nmid_d[cur], scale=1.0,
                                                     accum_out=cnt_d)
                                nc.vector.tensor_scalar(gejB, md8_t[:, NA:], mid_d[cur],
                                                        scalar2=1.0,
                                                        op0=mybir.AluOpType.is_ge,
                                                        op1=mybir.AluOpType.mult,
                                                        accum_out=cntB)
                                # sel = (cnt_dA + 2*cntB >= 2k - NA)
                                nc.vector.tensor_scalar_mul(selB, cntB, 2.0)
                                nc.vector.tensor_tensor(selB, selB, cnt_d,
                                                        mybir.AluOpType.add)
                                nc.vector.tensor_scalar(selB, selB,
                                                        float(2 * DEAD_TOPK - NA) - 0.5,
                                                        scalar2=None,
                                                        op0=mybir.AluOpType.is_ge)
                                nc.vector.tensor_scalar(stB, selB, float(2.0 * step_d),
                                                        scalar2=float(-step_d),
                                                        op0=mybir.AluOpType.mult,
                                                        op1=mybir.AluOpType.add)
                                nc.vector.tensor_tensor(mid_d[nxt], mid_d[cur], stB,
                                                        mybir.AluOpType.add)
                                nc.vector.tensor_scalar_mul(nmid_d[nxt], mid_d[nxt], -1.0)
                            else:
                                nc.scalar.activation(junk_d, md8_t, SIGN,
                                                     bias=nmid_d[cur], scale=1.0,
                                                     accum_out=cnt_d)
                                nc.scalar.activation(dir_d, cnt_d, SIGN,
                                                     bias=cb_d, scale=1.0)
                                nc.scalar.activation(nmid_d[nxt], dir_d, IDENT,
                                                     bias=nmid_d[cur], scale=-step_d)
                        fin = bis_iters % 2
                        t_m = bis.tile([128, 1], F32, name=f"t_m{rt}")
                        t_d = bis.tile([128, 1], F32, name=f"t_d{rt}")
                        nc.scalar.activation(t_m, nmid_m[fin], IDENT,
                                             bias=cw_m, scale=-1.0)
                        if split:
                            nc.vector.tensor_scalar(t_d, mid_d[fin],
                                                    float(-(w0_d / (2.0 ** bis_iters))),
                                                    scalar2=None, op0=mybir.AluOpType.add)
                        else:
                            nc.scalar.activation(t_d, nmid_d[fin], IDENT,
                                                 bias=cw_d, scale=-1.0)
                        nc.sync.dma_start(t_dram[0, rt * 128:(rt + 1) * 128], t_m)
                        nc.sync.dma_start(t_dram[1, rt * 128:(rt + 1) * 128], t_d)

                # constants for phase 2
                bias_b = const_pool.tile([128, E], F32, name="bias_b")
                nc.sync.dma_start(bias_b, bcast(bias_row[:, :]))
                pen_cols = const_pool.tile([128, f_total // 128], F32, name="pen_cols")
                nc.sync.dma_start(pen_cols, pen_pt[:, :])

                for pr in range(2 if "p2" in phases else 0):
                    b0 = pr * 256
                    tm4 = p2c.tile([128, ft_fuse, 256], F32, name="tm4")
                    td4 = p2c.tile([128, ft_fuse, 256], F32, name="td4")
                    for c in range(ft_fuse):
                        nc.sync.dma_start(tm4[:, c, :], bcast(t_dram[0:1, b0:b0 + 256]))
                        nc.sync.dma_start(td4[:, c, :], bcast(t_dram[1:2, b0:b0 + 256]))

                    pm = [ps2.tile([128, 512], F32, name=f"pm{j}", tag=f"pm{j}") for j in range(4)]
                    pd = [ps2.tile([128, 512], F32, name=f"pd{j}", tag=f"pd{j}") for j in range(4)]

                    for i64 in range(n_it64):
                        f0 = i64 * ft_fuse * 128
                        lk4 = p2.tile([128, ft_fuse, E], BF16, name="lk4")
                        src = lookup_bf[f0:f0 + ft_fuse * 128, :].rearrange(
                            "(c p) e -> p c e", p=128)
                        nc.gpsimd.dma_start(lk4[:, :, 0:768], src[:, :, 0:768])
                        nc.sync.dma_start(lk4[:, :, 768:], src[:, :, 768:])
                        pt4 = p2.tile([128, ft_fuse, 256], F32, name="pt4")
                        nc.sync.dma_start(
                            pt4, projT_dram[f0:f0 + ft_fuse * 128, b0:b0 + 256].rearrange(
                                "(c p) b -> p c b", p=128))

                        # main masks from the clean proj, then pen-add IN PLACE
                        # (pt4 not needed raw afterwards) for the dead masks
                        km4 = p2.tile([128, ft_fuse, 256], BF16, name="km4")
                        nc.vector.tensor_tensor(km4, pt4, tm4, mybir.AluOpType.is_ge)
                        smain = p2.tile([128, ft_fuse, 256], BF16, name="smain")
                        nc.vector.tensor_tensor(smain, pt4, km4, mybir.AluOpType.mult)
                        for c in range(ft_fuse):
                            nc.vector.tensor_scalar(
                                pt4[:, c, :], pt4[:, c, :],
                                pen_cols[:, i64 * ft_fuse + c: i64 * ft_fuse + c + 1],
                                scalar2=None, op0=mybir.AluOpType.add)
                        kd4 = p2.tile([128, ft_fuse, 256], BF16, name="kd4")
                        nc.vector.tensor_tensor(kd4, pt4, td4, mybir.AluOpType.is_ge)
                        sdead = p2.tile([128, ft_fuse, 256], BF16, name="sdead")
                        nc.vector.tensor_tensor(sdead, pt4, kd4, mybir.AluOpType.mult)

                        for c in range(ft_fuse):
                            st = (i64 == 0 and c == 0)
                            sp = (i64 == n_it64 - 1 and c == ft_fuse - 1)
                            for bs in range(2):
                                for eh in range(2):
                                    j = bs * 2 + eh
                                    nc.tensor.matmul(
                                        pm[j], smain[:, c, bs * 128:(bs + 1) * 128],
                                        lk4[:, c, eh * 512:(eh + 1) * 512],
                                        start=st, stop=sp)
                                    nc.tensor.matmul(
                                        pd[j], sdead[:, c, bs * 128:(bs + 1) * 128],
                                        lk4[:, c, eh * 512:(eh + 1) * 512],
                                        start=st, stop=sp)

                    for bs in range(2):
                        for eh in range(2):
                            j = bs * 2 + eh
                            om = p2o.tile([128, 512], F32, name=f"om{j}")
                            nc.vector.tensor_tensor(om, pm[j], bias_b[:, eh * 512:(eh + 1) * 512],
                                                    mybir.AluOpType.add)
                            nc.scalar.dma_start(
                                out_main[b0 + bs * 128:b0 + (bs + 1) * 128,
                                         eh * 512:(eh + 1) * 512], om)
                            od = p2o.tile([128, 512], F32, name=f"od{j}")
                            nc.vector.tensor_scalar(od, pd[j], 0.0, scalar2=None,
                                                    op0=mybir.AluOpType.add)
                            nc.scalar.dma_start(
                                out_dead[b0 + bs * 128:b0 + (bs + 1) * 128,
                                         eh * 512:(eh + 1) * 512], od)

    nc.finalize()
    return nc


def _split_fp16(a):
    """fp32 -> (hi, lo) fp16 pair with a = hi + lo*2^-12 to ~23 mantissa bits.

    Values below the fp16 min-normal go wholly into the (scaled) lo part so
    the PE never sees fp16 subnormals in the hi product.
    """
    hi = a.astype(np.float16)
    hi[np.abs(a) < 6.104e-5] = np.float16(0.0)
    lo = ((a - hi.astype(np.float32)) * 4096.0).astype(np.float16)
    return np.ascontiguousarray(hi), np.ascontiguousarray(lo)


def get_nc():
    if F not in _CACHED:
        _CACHED[F] = _build(F)
    return _CACHED[F]


def prep_in_maps(embed, enc_bias, enc_W, lookup_np, usage):
    import ml_dtypes

    x = embed - enc_bias[None, :]
    xT = np.ascontiguousarray(x.T)              # [E, B]
    xhT, xlT = _split_fp16(xT)
    WT = np.ascontiguousarray(enc_W.T)          # [E, F]
    whT, wlT = _split_fp16(WT)
    lookup_bf = np.ascontiguousarray(lookup_np.astype(ml_dtypes.bfloat16))
    pen = np.where(usage > DEAD_CUTOFF, np.float32(0.0), np.float32(-1e30)).astype(np.float32)
    pen_row = pen.reshape(1, F)
    pen_pt = np.ascontiguousarray(pen.reshape(F // 128, 128).T)  # [128, F//128]
    bias_row = enc_bias.reshape(1, E)

    in_maps = []
    for c in range(NCORES):
        sl = slice(c * BL, (c + 1) * BL)
        in_maps.append({
            "whT": whT, "wlT": wlT,
            "xhT": np.ascontiguousarray(xhT[:, sl]),
            "xlT": np.ascontiguousarray(xlT[:, sl]),
            "lookup_bf": lookup_bf,
            "pen_row": pen_row, "pen_pt": pen_pt, "bias_row": bias_row,
        })
    return in_maps


def postprocess(res, n_cores=NCORES):
    return res["out_main"], res["out_dead"]


def kernel(embed, enc_bias, enc_W, lookup, last_usage):
    from concourse.bass_utils import run_bass_kernel_spmd

    embed = np.asarray(embed, dtype=np.float32)
    enc_bias = np.asarray(enc_bias, dtype=np.float32)
    enc_W = np.asarray(enc_W, dtype=np.float32)
    lookup_np = np.ascontiguousarray(np.asarray(lookup, dtype=np.float32))
    usage = np.asarray(last_usage)

    in_maps = prep_in_maps(embed, enc_bias, enc_W, lookup_np, usage)
    nc = get_nc()

    res = run_bass_kernel_spmd(nc, in_maps, core_ids=list(range(NCORES)))
    globals()["LAST_RES"] = res
    er = np.concatenate([res.results[c]["out_main"] for c in range(NCORES)], axis=0)
    dr = np.concatenate([res.results[c]["out_dead"] for c in range(NCORES)], axis=0)
    return er, dr

